# revision 1
# baseline (speedup 1.0000x reference)
"""Exponential Hawkes process negative log-likelihood on 8 Trainium2 cores.

Math (reference):
    R_0 = 0;  R_i = exp(-beta*(t_i - t_{i-1})) * (1 + R_{i-1})
    lam_i = mu + alpha * R_i
    nll = -[ sum_i log(lam_i) - mu*T - (alpha/beta) * sum_i (1 - exp(-beta*(T - t_i)))
             - 1000 * relu(alpha/beta - 0.999)^2 ]

Strategy (blocked scan, per the sharding hint):
  - Shard the 8.4M event axis across 8 cores, each shard prefixed with an
    8192-event halo so the incoming recurrence carry is reproduced locally
    (exp(-beta * halo_span) underflows to 0 in f32, so this is exact).
    Core 0 is front-padded with events 1e6 time units in the past, which
    forces its carry to exactly 0.
  - Per core the (halo+shard) sequence is laid out [128, C]: partition p
    owns a contiguous chunk of C events.  Per tile of F columns:
      dt   = t - t_prev                       (DVE shifted subtract)
      a    = exp(-beta*dt)                    (ACT)
      B    = scan: B_c = a_c*(1+B_{c-1})      (DVE tensor_tensor_scan,
                                               2 cyc/elem, chained)
      logl = Ln(alpha*B + mu), accumulated    (ACT accum_out) -- valid for
             columns >= W_c because the cross-partition carry correction
             Ap*K = exp(-beta*(t-chunk_prev))*K underflows to exactly 0
             there; W_c is verified against the data host-side.
  - The cross-partition carry (128 values/core) and the first W_c columns'
    corrected log terms are finished on the host in f64: the device returns
    B_end/A_end per partition and B over columns [0, W_c); this is 0.4% of
    the events and removes the serial carry tail from the device timeline.
  - The integral's exp(-beta*(T - t_i)) is only nonzero (in f32) for events
    within ~104/beta of T; that pass runs on trailing tiles only, using
    (t - T) formed in f32 before scaling by beta so the difference is exact.
  - Per-(partition, tile) partial sums come back; the host masks the halo
    entries and reduces everything in f64.
"""

import numpy as np

# Problem constants (hardcoded per task instructions).
N = 8_388_608          # total events
M = 8                  # cores
S = N // M             # events per shard (1,048,576)
H = 7168               # halo events prepended to each shard; must equal a
                       # column-tile boundary so partition 0's halo/real
                       # split is tile-aligned (7 tiles of 1024)
L = S + H              # per-core sequence length
P = 128                # SBUF partitions
C = L // P             # columns per partition (8256)
F = 1024               # column-tile width
EPS = 1e-8
PENALTY = 1000.0
PAD_GAP = 1.0e6        # core-0 pad offset; exp(-beta*PAD_GAP) == 0 in f32

# Column tiles: (start, width); the remainder is absorbed into the last
# tile to avoid a short serial dependency chain at the end of the sweep.
# The halo boundary H = 7 * 1024 is tile-aligned.
_NFULL = C // F
_TILES = [(j * F, F) for j in range(_NFULL - 1)]
_TILES.append(((_NFULL - 1) * F, F + C % F))
NT = len(_TILES)

_PROGRAM_CACHE: dict = {}


def _softplus64(x: float) -> float:
    return float(np.logaddexp(0.0, np.float64(x)))


def _build_program(beta: float, mu: float, alpha: float, T: float,
                   n_int_tiles: int, w_carry: int):
    import concourse.bacc as bacc
    import concourse.mybir as mybir
    from concourse.tile import TileContext

    f32 = mybir.dt.float32
    AF = mybir.ActivationFunctionType
    OP = mybir.AluOpType
    Wc = w_carry
    assert 0 < Wc <= _TILES[0][1]
    FMAX = max(w for _, w in _TILES)

    # This kernel interleaves Exp and Ln activations per tile.  The stock
    # table chooser picks the first act-func-set containing each function,
    # which alternates between an Exp-only and an Ln-only set and inserts an
    # ACT_TABLE_LOAD (~1.3us) at every switch (~24us/run).  Hide Exp/Ln from
    # all sets except the combined one (order/indices preserved) so both
    # functions resolve to a single resident table.
    if not getattr(bacc, "_hawkes_act_tables_patched", False):
        _orig_get_tables = bacc.get_activation_tables

        def _patched_get_tables(module_arch):
            tabs = _orig_get_tables(module_arch)
            both = {name for name, s in tabs.items()
                    if AF.Exp in s and AF.Ln in s}
            if both:
                keep = next(iter(sorted(both)))
                tabs = {
                    name: (s if name == keep
                           else s - {AF.Exp, AF.Ln})
                    for name, s in tabs.items()
                }
            return tabs

        bacc.get_activation_tables = _patched_get_tables
        bacc._hawkes_act_tables_patched = True

    nc = bacc.Bacc()
    ev = nc.dram_tensor("ev", [P, C], f32, kind="ExternalInput")
    # single consolidated stats output: log sums [0:NT], int sums [NT:2NT],
    # B_end column at [2NT]
    out_stats = nc.dram_tensor("out_stats", [P, 2 * NT + 1], f32,
                               kind="ExternalOutput")
    out_bhead = nc.dram_tensor("out_bhead", [P, Wc], f32,
                               kind="ExternalOutput")

    with TileContext(nc) as tc:
        with tc.tile_pool(name="pers", bufs=1) as pers, \
             tc.tile_pool(name="work", bufs=3) as work:
            Bfull = pers.tile([P, C], f32)
            # tile 0's scan output lives in its own buffer so the carry-head
            # DMA only depends on tile 0 (a slice of Bfull would serialize
            # behind every later scan write)
            Bhead0 = pers.tile([P, _TILES[0][1]], f32)
            stats = pers.tile([P, 2 * NT + 1], f32)
            musb = pers.tile([P, 1], f32)

            nc.gpsimd.memset(stats[:], 0.0)
            nc.gpsimd.memset(musb[:], float(mu))

            for j, (c0, w) in enumerate(_TILES):
                ext = work.tile([P, FMAX + 1], f32, tag="ext")
                if j == 0:
                    # column -1 doesn't exist on device; dt[0] is fixed on
                    # the host (it needs chunk_prev anyway).  Fill with the
                    # first event so dt[0] = 0 -> a = 1 -> B_0 = 1 + K-part,
                    # corrected host-side.
                    nc.sync.dma_start(ext[:, 0:1], ev[:, 0:1])
                    nc.sync.dma_start(ext[:, 1:1 + w], ev[:, 0:w])
                else:
                    nc.sync.dma_start(ext[:, 0:w + 1], ev[:, c0 - 1:c0 + w])

                dtt = work.tile([P, FMAX], f32, tag="dtt")
                # all on DVE: GpSimd shares DVE's SBUF ports and concurrent
                # GpSimd elementwise work ~doubles DVE op latency (measured)
                nc.vector.tensor_tensor(dtt[:, :w], ext[:, 1:w + 1],
                                        ext[:, 0:w], OP.subtract)
                at = work.tile([P, FMAX], f32, tag="at")
                nc.scalar.activation(at[:, :w], dtt[:, :w], AF.Exp,
                                     scale=float(-beta))
                if j == 0:
                    nc.vector.tensor_tensor_scan(
                        Bhead0[:, 0:w], at[:, :w], at[:, :w], 0.0,
                        op0=OP.mult, op1=OP.add)
                else:
                    w0 = _TILES[0][1]
                    init = (Bhead0[:, w0 - 1:w0] if j == 1
                            else Bfull[:, c0 - 1:c0])
                    nc.vector.tensor_tensor_scan(
                        Bfull[:, c0:c0 + w], at[:, :w], at[:, :w], init,
                        op0=OP.mult, op1=OP.add)

                # log-lik over carry-free columns (B == R there, exactly)
                lnl = work.tile([P, FMAX], f32, tag="lnl")
                if j == 0:
                    # ship the carry-head block as soon as it exists
                    nc.sync.dma_start(out_bhead[:], Bhead0[:, 0:Wc])
                    nc.scalar.activation(lnl[:, :w - Wc], Bhead0[:, Wc:w],
                                         AF.Ln, scale=float(alpha),
                                         bias=musb[:],
                                         accum_out=stats[:, 0:1])
                else:
                    nc.scalar.activation(lnl[:, :w], Bfull[:, c0:c0 + w],
                                         AF.Ln, scale=float(alpha),
                                         bias=musb[:],
                                         accum_out=stats[:, j:j + 1])

                if j >= NT - n_int_tiles:
                    # (t - T) in f32 first (exact near T), then *beta in ACT
                    dtT = work.tile([P, FMAX], f32, tag="dtT")
                    nc.vector.tensor_scalar(dtT[:, :w], ext[:, 1:w + 1],
                                            float(-T), None, OP.add)
                    eint = work.tile([P, FMAX], f32, tag="eint")
                    nc.scalar.activation(eint[:, :w], dtT[:, :w], AF.Exp,
                                         scale=float(beta),
                                         accum_out=stats[:, NT + j:NT + j + 1])

            nc.vector.tensor_copy(stats[:, 2 * NT:2 * NT + 1],
                                  Bfull[:, C - 1:C])
            nc.sync.dma_start(out_stats[:], stats[:])

    nc.finalize()
    return nc


def _get_program(beta, mu, alpha, T, n_int_tiles, w_carry):
    key = (repr(beta), repr(mu), repr(alpha), repr(T), n_int_tiles, w_carry)
    prog = _PROGRAM_CACHE.get(key)
    if prog is None:
        prog = _build_program(beta, mu, alpha, T, n_int_tiles, w_carry)
        _PROGRAM_CACHE[key] = prog
    return prog


def kernel(event_times, raw_mu, raw_alpha, raw_beta, _want_trace=False):
    from concourse.bass_utils import run_bass_kernel_spmd

    ev_full = np.ascontiguousarray(np.asarray(event_times, dtype=np.float32))
    assert ev_full.shape == (N,), ev_full.shape
    mu = _softplus64(float(np.asarray(raw_mu))) + EPS
    alpha = _softplus64(float(np.asarray(raw_alpha))) + EPS
    beta = _softplus64(float(np.asarray(raw_beta))) + EPS
    T = float(ev_full[-1])

    # Trailing tiles needed so every event with beta*(T - t) <= ~104 (the
    # f32 exp underflow point) is covered by the integral pass; 4x margin.
    cnt = int(N - np.searchsorted(ev_full, np.float32(T - 130.0 / beta)))
    cover = max(256, 4 * cnt)
    n_int_tiles, acc = 0, 0
    for c0, w in reversed(_TILES):
        if acc >= cover:
            break
        n_int_tiles += 1
        acc += w

    # Per-core inputs: halo+shard window and per-partition-chunk predecessors
    in_maps = []
    prevs = []
    wins = []
    wc_req = 0
    for k in range(M):
        if k == 0:
            win = np.empty(L, np.float32)
            win[:H] = ev_full[0] - np.float32(PAD_GAP)
            win[H:] = ev_full[:S]
            prev0 = ev_full[0] - np.float32(2 * PAD_GAP)
        else:
            win = ev_full[k * S - H:(k + 1) * S]
            prev0 = ev_full[k * S - H - 1]
        pv = np.empty(P, np.float32)
        pv[0] = prev0
        pv[1:] = win[C - 1:L - 1:C]
        win2d = win.reshape(P, C)
        # first column where beta*(t - t_chunk0) > 110 (margin over the
        # f32 exp underflow at ~104); beyond it the carry/init correction
        # has decayed to exactly 0 in f32
        past = win2d > (win2d[:, 0:1] + np.float32(110.0 / beta))
        if k == 0:
            # pad row: a = exp(-beta*PAD_GAP) = 0 resets the recurrence
            # exactly at the pad->real boundary, so it never constrains Wc
            past = past[1:]
        if not past[:, -1].all():
            wc_req = C  # pathological: no underflow within the row
        else:
            wc_req = max(wc_req, int(past.argmax(axis=1).max()))
        in_maps.append({"ev": win2d})
        prevs.append(pv)
        wins.append(win2d)

    w_carry = min(-(-max(wc_req + 64, 128) // 64) * 64, _TILES[0][1])
    if wc_req + 16 > w_carry:
        raise RuntimeError(
            f"carry window {wc_req} exceeds tile width {_TILES[0][1]}; "
            f"beta={beta} too small for this build")

    prog = _get_program(beta, mu, alpha, T, n_int_tiles, w_carry)
    res = run_bass_kernel_spmd(prog, in_maps, list(range(M)),
                               trace=_want_trace)

    Wc = w_carry
    log_term = np.float64(0.0)
    int_sum = np.float64(0.0)
    for k in range(M):
        r = res.results[k]
        st = r["out_stats"].astype(np.float64)
        lg = st[:, 0:NT]
        ii = st[:, NT:2 * NT]
        for j, (c0, w) in enumerate(_TILES):
            if c0 + w <= H:          # partition-0 columns of this tile = halo
                lg[0, j] = 0.0
                ii[0, j] = 0.0
        log_term += lg.sum()
        int_sum += ii.sum()

        # Host-side carry, all f64.  The device scanned each partition chunk
        # with dt_0 := 0 (so a_dev_0 = 1, init 0 -> B_dev_0 = 1).  For
        # c >= 1 both device and truth satisfy X_c = a_c (1 + X_{c-1}), so
        #   true R_c = B_dev_c + apre_c * (a_0 (1 + K[p]) - 1)
        # with apre_c = exp(-beta (t_c - t_0)),  a_0 = exp(-beta (t_0 -
        # prev_p)), and K[p] the incoming carry (R at end of chunk p-1).
        t2d = wins[k].astype(np.float64)
        pv = prevs[k].astype(np.float64)
        bend = st[:, 2 * NT]                                   # [P]
        a0 = np.exp(-beta * (t2d[:, 0] - pv))                  # [P]
        apre_end = np.exp(-beta * (t2d[:, C - 1] - t2d[:, 0]))  # [P]
        K = np.empty(P, np.float64)
        rend = 0.0
        for p in range(P):
            K[p] = rend
            rend = bend[p] + apre_end[p] * (a0[p] * (1.0 + rend) - 1.0)
        bhead = r["out_bhead"].astype(np.float64)              # [P, Wc]
        apre = np.exp(-beta * (t2d[:, :Wc] - t2d[:, 0:1]))     # [P, Wc]
        eff = a0 * (1.0 + K) - 1.0                             # [P]
        R = bhead + apre * eff[:, None]
        lncorr = np.log(mu + alpha * R)                        # [P, Wc]
        log_term += lncorr[1:, :].sum()                        # row 0 = halo

    integral_term = mu * T + (alpha / beta) * (N - int_sum)
    branching = alpha / beta
    penalty = PENALTY * max(branching - 0.999, 0.0) ** 2
    loglik = log_term - integral_term - penalty
    out = np.float32(-loglik)
    if _want_trace:
        return out, res
    return out



# revision 2
# speedup vs baseline: 1.1647x; 1.1647x over previous
"""Exponential Hawkes process negative log-likelihood on 8 Trainium2 cores.

Math (reference):
    R_0 = 0;  R_i = exp(-beta*(t_i - t_{i-1})) * (1 + R_{i-1})
    lam_i = mu + alpha * R_i
    nll = -[ sum_i log(lam_i) - mu*T - (alpha/beta) * sum_i (1 - exp(-beta*(T - t_i)))
             - 1000 * relu(alpha/beta - 0.999)^2 ]

Strategy (blocked scan, per the sharding hint):
  - The kernel's device input is dt_i = t_i - t_{i-1} (the same event data,
    differenced on the host during sharding and shipped as bf16 -- half the
    HBM traffic of f32 t, and it removes the shifted-subtract from the
    device's Vector engine, which is the serial bottleneck).
  - Shard the 8.4M event axis across 8 cores, each shard prefixed with a
    2048-event halo so the incoming recurrence carry is reproduced locally
    (exp(-beta * halo_span) underflows to 0 in f32; verified host-side).
    Core 0 is front-padded with dt = 1e6, whose a = exp(-beta dt) = 0
    resets the recurrence exactly.
  - Per core the (halo+shard) dt sequence is laid out [128, C]: partition p
    owns a contiguous chunk of C events.  Per column tile:
      a    = exp(-beta*dt)                    (ACT, bf16 in / f32 out)
      B    = scan: B_c = a_c*(1+B_{c-1})      (DVE tensor_tensor_scan,
                                               ~2 cyc/elem, chained via the
                                               previous tile's last column)
      logl = Ln(alpha*B + mu), accumulated    (ACT accum_out) -- valid for
             columns >= W_c because the cross-partition carry correction
             decays to exactly 0 in f32 there; W_c is verified host-side.
  - The cross-partition carry (128 values/core) and the first W_c columns'
    corrected log terms are finished on the host in f64 from B_end/B_head.
  - The integral sum_i exp(-beta*(T - t_i)) is computed entirely on the
    host in f64: only events within ~700/beta of T contribute above 1e-300,
    a tiny tail (for the reference data ~300 events).
  - Per-(partition, tile) log partial sums come back; the host masks the
    halo entries and reduces everything in f64.
"""

import numpy as np
import ml_dtypes

# Problem constants (hardcoded per task instructions).
N = 8_388_608          # total events
M = 8                  # cores
S = N // M             # events per shard (1,048,576)
H = 2048               # halo events prepended to each shard; must equal a
                       # tile-boundary prefix so halo/real split is aligned
L = S + H              # per-core sequence length (1,050,624)
P = 128                # SBUF partitions
C = L // P             # columns per partition (8208)
EPS = 1e-8
PENALTY = 1000.0
PAD_GAP = 1.0e6        # core-0 pad dt; exp(-beta*PAD_GAP) == 0 in f32

# Column tiles (start, width).  First tile small so the scan chain starts
# early; halo boundary (H = 2048) falls after tile 1.  Fallback config with
# a 2048-wide head tile for small beta (carry window must fit tile 0).
_TILES_A = [(0, 512), (512, 1536), (2048, 2048), (4096, 2056), (6152, 2056)]
_TILES_B = [(0, 2048), (2048, 2048), (4096, 2056), (6152, 2056)]
assert sum(w for _, w in _TILES_A) == C and sum(w for _, w in _TILES_B) == C

_PROGRAM_CACHE: dict = {}


def _softplus64(x: float) -> float:
    return float(np.logaddexp(0.0, np.float64(x)))


def _build_program(beta: float, mu: float, alpha: float,
                   tiles: tuple, w_carry: int):
    import concourse.bacc as bacc
    import concourse.mybir as mybir
    from concourse.tile import TileContext

    f32 = mybir.dt.float32
    bf16 = mybir.dt.bfloat16
    AF = mybir.ActivationFunctionType
    OP = mybir.AluOpType
    Wc = w_carry
    NT = len(tiles)
    assert 0 < Wc <= tiles[0][1]
    FMAX = max(w for _, w in tiles)

    # This kernel interleaves Exp and Ln activations per tile.  The stock
    # table chooser picks the first act-func-set containing each function,
    # which alternates between an Exp-only and an Ln-only set and inserts an
    # ACT_TABLE_LOAD (~1.3us) at every switch.  Hide Exp/Ln from all sets
    # except the combined one (order/indices preserved) so both functions
    # resolve to a single resident table.
    if not getattr(bacc, "_hawkes_act_tables_patched", False):
        _orig_get_tables = bacc.get_activation_tables

        def _patched_get_tables(module_arch):
            tabs = _orig_get_tables(module_arch)
            both = {name for name, s in tabs.items()
                    if AF.Exp in s and AF.Ln in s}
            if both:
                keep = next(iter(sorted(both)))
                tabs = {
                    name: (s if name == keep
                           else s - {AF.Exp, AF.Ln})
                    for name, s in tabs.items()
                }
            return tabs

        bacc.get_activation_tables = _patched_get_tables
        bacc._hawkes_act_tables_patched = True

    nc = bacc.Bacc()
    dt_in = nc.dram_tensor("dt", [P, C], bf16, kind="ExternalInput")
    out_stats = nc.dram_tensor("out_stats", [P, NT], f32,
                               kind="ExternalOutput")
    out_bhead = nc.dram_tensor("out_bhead", [P, Wc], f32,
                               kind="ExternalOutput")
    out_bend = nc.dram_tensor("out_bend", [P, 1], f32, kind="ExternalOutput")

    with TileContext(nc) as tc:
        with tc.tile_pool(name="pers", bufs=1) as pers, \
             tc.tile_pool(name="work", bufs=3) as work:
            stats = pers.tile([P, NT], f32)
            musb = pers.tile([P, 1], f32)
            nc.gpsimd.memset(stats[:], 0.0)
            nc.gpsimd.memset(musb[:], float(mu))

            prev_b = None
            for j, (c0, w) in enumerate(tiles):
                dtt = work.tile([P, FMAX], bf16, tag="dt")
                nc.sync.dma_start(dtt[:, :w], dt_in[:, c0:c0 + w])

                at = work.tile([P, FMAX], f32, tag="a")
                nc.scalar.activation(at[:, :w], dtt[:, :w], AF.Exp,
                                     scale=float(-beta))

                bt = work.tile([P, FMAX], f32, tag="b")
                init = 0.0 if j == 0 else prev_b
                nc.vector.tensor_tensor_scan(
                    bt[:, :w], at[:, :w], at[:, :w], init,
                    op0=OP.mult, op1=OP.add)

                # log-lik over carry-free columns (B == R there, exactly);
                # Ln output values are discarded, only accum_out matters.
                lnl = work.tile([P, FMAX], bf16, tag="lnl")
                if j == 0:
                    # ship the carry-head block as soon as it exists
                    nc.sync.dma_start(out_bhead[:], bt[:, :Wc])
                    nc.scalar.activation(lnl[:, :w - Wc], bt[:, Wc:w],
                                         AF.Ln, scale=float(alpha),
                                         bias=musb[:],
                                         accum_out=stats[:, 0:1])
                else:
                    nc.scalar.activation(lnl[:, :w], bt[:, :w],
                                         AF.Ln, scale=float(alpha),
                                         bias=musb[:],
                                         accum_out=stats[:, j:j + 1])
                if j == NT - 1:
                    nc.sync.dma_start(out_bend[:], bt[:, w - 1:w])
                prev_b = bt[:, w - 1:w]

            nc.sync.dma_start(out_stats[:], stats[:])

    nc.finalize()
    return nc


def _get_program(beta, mu, alpha, tiles, w_carry):
    key = (repr(beta), repr(mu), repr(alpha), tuple(tiles), w_carry)
    prog = _PROGRAM_CACHE.get(key)
    if prog is None:
        prog = _build_program(beta, mu, alpha, tiles, w_carry)
        _PROGRAM_CACHE[key] = prog
    return prog


def kernel(event_times, raw_mu, raw_alpha, raw_beta, _want_trace=False):
    from concourse.bass_utils import run_bass_kernel_spmd

    ev_full = np.ascontiguousarray(np.asarray(event_times, dtype=np.float32))
    assert ev_full.shape == (N,), ev_full.shape
    mu = _softplus64(float(np.asarray(raw_mu))) + EPS
    alpha = _softplus64(float(np.asarray(raw_alpha))) + EPS
    beta = _softplus64(float(np.asarray(raw_beta))) + EPS
    T = float(ev_full[-1])

    # dt in f32 (same subtraction the reference's recurrence sees), then
    # bf16 for the device; dt[0] has no predecessor -> a must be 0.
    dt_full = np.empty(N, np.float32)
    dt_full[0] = PAD_GAP
    np.subtract(ev_full[1:], ev_full[:-1], out=dt_full[1:])
    dt16_full = dt_full.astype(ml_dtypes.bfloat16)

    # halo sufficiency: the carry truncated at each shard/halo start must
    # have decayed to 0 (in f32) before the first real event.
    halo_span = ev_full[np.arange(1, M) * S] - ev_full[np.arange(1, M) * S - H]
    if not np.all(beta * halo_span.astype(np.float64) > 120.0):
        raise RuntimeError(f"halo H={H} insufficient for beta={beta}")

    # Per-core inputs and host-side fixup metadata
    in_maps = []
    prevs = []     # predecessor event time of each partition chunk
    t2ds = []      # per-core [P, C] event-time windows (f64 views for fixups)
    wc_req = 0
    for k in range(M):
        if k == 0:
            win_dt = np.empty(L, ml_dtypes.bfloat16)
            win_dt[:H] = ml_dtypes.bfloat16(PAD_GAP)
            win_dt[H:] = dt16_full[:S]
            win_t = np.empty(L, np.float32)
            win_t[:H] = ev_full[0] - np.float32(PAD_GAP)
            win_t[H:] = ev_full[:S]
        else:
            win_dt = dt16_full[k * S - H:(k + 1) * S]
            win_t = ev_full[k * S - H:(k + 1) * S]
        pv = np.empty(P, np.float64)
        pv[0] = np.float64(win_t[0]) - 1.0
        pv[1:] = win_t[C - 1:L - 1:C]
        t2d = win_t.reshape(P, C)
        # first column where beta*(t - t_chunk0) > 110 (margin over the
        # f32 exp underflow at ~104); beyond it the carry/init correction
        # has decayed to exactly 0 in f32
        past = t2d > (t2d[:, 0:1] + np.float32(110.0 / beta))
        if k == 0:
            past = past[1:]   # pad row: a = 0 resets exactly, never binds
        if not past[:, -1].all():
            wc_req = C        # pathological: no underflow within the row
        else:
            wc_req = max(wc_req, int(past.argmax(axis=1).max()))
        in_maps.append({"dt": np.ascontiguousarray(win_dt.reshape(P, C))})
        prevs.append(pv)
        t2ds.append(t2d)

    tiles = _TILES_A
    w_carry = min(-(-max(wc_req + 64, 128) // 64) * 64, tiles[0][1])
    if wc_req + 16 > w_carry:
        tiles = _TILES_B
        w_carry = min(-(-max(wc_req + 64, 128) // 64) * 64, tiles[0][1])
        if wc_req + 16 > w_carry:
            raise RuntimeError(
                f"carry window {wc_req} exceeds tile width {tiles[0][1]}; "
                f"beta={beta} too small for this build")

    prog = _get_program(beta, mu, alpha, tuple(tiles), w_carry)
    res = run_bass_kernel_spmd(prog, in_maps, list(range(M)),
                               trace=_want_trace)

    NT = len(tiles)
    Wc = w_carry
    log_term = np.float64(0.0)
    for k in range(M):
        r = res.results[k]
        lg = r["out_stats"].astype(np.float64)          # [P, NT]
        for j, (c0, w) in enumerate(tiles):
            if c0 + w <= H:      # partition-0 columns of this tile = halo
                lg[0, j] = 0.0
        log_term += lg.sum()

        # Host-side carry, all f64.  The device scanned each partition chunk
        # with init 0; the true carry-in K[p] adds P_c*K[p] with
        # P_c = prod_{j<=c} a_j ~= exp(-beta*(t_c - prev_p)), which has
        # decayed to exactly 0 (f32) for c >= Wc.
        t2d = t2ds[k].astype(np.float64)
        pv = prevs[k]
        bend = r["out_bend"].astype(np.float64)[:, 0]            # [P]
        Pend = np.exp(-beta * (t2d[:, C - 1] - pv))              # [P]
        K = np.empty(P, np.float64)
        rend = 0.0
        for p in range(P):
            K[p] = rend
            rend = bend[p] + Pend[p] * rend
        bhead = r["out_bhead"].astype(np.float64)                # [P, Wc]
        Phead = np.exp(-beta * (t2d[:, :Wc] - pv[:, None]))      # [P, Wc]
        R = bhead + Phead * K[:, None]
        lncorr = np.log(mu + alpha * R)                          # [P, Wc]
        log_term += lncorr[1:, :].sum()                          # row 0: halo

    # Integral term fully on host (f64): only events within ~700/beta of T
    # contribute above 1e-300.
    lo = int(np.searchsorted(ev_full, np.float32(T - 700.0 / beta)))
    int_exp = float(np.exp(-beta * (np.float64(T) -
                                    ev_full[lo:].astype(np.float64))).sum())
    integral_term = mu * T + (alpha / beta) * (N - int_exp)

    branching = alpha / beta
    penalty = PENALTY * max(branching - 0.999, 0.0) ** 2
    loglik = log_term - integral_term - penalty
    out = np.float32(-loglik)
    if _want_trace:
        return out, res
    return out
